# revision 18
# baseline (speedup 1.0000x reference)
"""MoE routing kernel (nn_JSMLP): per-row expert-indexed 3-layer MLP.

  out[n] = Wl[i] @ tanh(W2[i] @ tanh(W1[i] @ x[n] + b1[i]) + b2[i]) + bl[i],  i = ind[n]

Sharding (hardcoded): expert-parallel across 8 cores, load-balanced by
count-sorted round-robin so a single SPMD program fits all cores:
experts are sorted by row count (desc); rank r goes to core r%8, slot r//8.
Slot s then has the same capacity cap[s] = roundup(count of rank 8s, 4) on
every core, so per-slot column spans are compile-time constants while padding
stays ~3% (vs ~50% for a uniform max-count capacity).

Numerics: W1 (with b1 row) and W2 are stored as float8_e3m4 scaled by 64
(values land in e3m4's normal range; 4 mantissa bits ~ 1.2% rms/elem); the
1/64 descale rides the tanh activations for free (out = tanh(scale*in)).
Wl, x, h, biases stay bf16; PSUM accumulates fp32. Measured end-to-end rel
err ~1.7e-2 vs the fp32 reference (gate: 2e-2).

Per core, per block b (4 slots, S_b = sum of caps <= 512 = one PSUM bank):
  L1: H1T[256, S_b] = W1augT.T @ [x;1]T   (bias via ones-row, K=65, e3m4 x64)
  L2: H2T[256, S_b] = W2T.T @ tanh(H1T/64)  (K=256 in 2 chunks; b2 seeded by a
      tiny block-diag ones matmul in bf16, also x64)
  L3: outT[2x64, PS_b] = WlT.T @ tanh(H2T/64)  (2 experts per 128-partition
      tile; bl seeded via pair-diag ones; plain bf16)

DMA plan (the shared DMA engine pool is the roofline at ~360 B/ns): one load
each for consts/x/W1 (W1 split in 2), per-2-block loads for W2/Wl, per-2-block
stores on the DVE queue; 15 DMAs total keeps the serial HWDGE (~630ns each)
off the critical path. All loads are issued into resident SBUF tiles (no ring
reuse) so the load queue never blocks on consumers.
"""

import numpy as np
import ml_dtypes

N, IN_DIM, H1, H2, LIN, NEXP = 16384, 64, 256, 256, 64, 256
NCORES = 8
SLOTS = NEXP // NCORES  # 32 experts per core

BF16 = ml_dtypes.bfloat16
E3M4 = ml_dtypes.float8_e3m4
WSCALE = 64.0

_cache = {}


def _geometry(caps):
    """Block/pair geometry shared by program builder and host prep.

    caps: per-slot capacities (len 32, multiples of 4, may be 0).
    Returns dict with blocks (list of slot-index lists), per-block slot
    offsets, block x-offsets, pair layout and output offsets.
    """
    blocks = []
    cur, cur_sum = [], 0
    for s in range(SLOTS):
        c = caps[s]
        if c == 0:
            continue
        if len(cur) == 8 or (cur_sum + c > 512 and cur):
            blocks.append(cur)
            cur, cur_sum = [], 0
        cur.append(s)
        cur_sum += c
    if cur:
        blocks.append(cur)
    # tiny trailing blocks shorten the drain after the last weight DMA
    if len(blocks[-1]) > 1:
        blocks.append([blocks[-1].pop()])
    if len(blocks[-2]) > 1:
        blocks.insert(len(blocks) - 1, [blocks[-2].pop()])

    g = {"blocks": blocks, "xoff": [], "S": [], "XO": [], "pairs": [],
         "poff": [], "PS": [], "OO": []}
    xo_total, oo_total = 0, 0
    for bl in blocks:
        offs, acc = [], 0
        for s in bl:
            offs.append(acc)
            acc += caps[s]
        assert acc <= 512, f"block span {acc} exceeds a PSUM bank"
        g["xoff"].append(offs)
        g["S"].append(acc)
        g["XO"].append(xo_total)
        xo_total += acc
        prs = [(bl[i], bl[i + 1] if i + 1 < len(bl) else None)
               for i in range(0, len(bl), 2)]
        poffs, pacc = [], 0
        for a, b in prs:
            poffs.append(pacc)
            pacc += max(caps[a], caps[b] if b is not None else 0)
        g["pairs"].append(prs)
        g["poff"].append(poffs)
        g["PS"].append(pacc)
        g["OO"].append(oo_total)
        oo_total += pacc
    g["TOT"] = xo_total
    g["TOT2"] = oo_total
    return g


def _build_program(caps):
    import concourse.bass as bass
    import concourse.tile as tile
    from concourse import bacc, mybir

    caps = list(caps)
    g = _geometry(caps)
    blocks, S, XO, xoff = g["blocks"], g["S"], g["XO"], g["xoff"]
    pairs, poff, PS, OO = g["pairs"], g["poff"], g["PS"], g["OO"]
    NB = len(blocks)
    TOT, TOT2 = g["TOT"], g["TOT2"]

    f32 = mybir.dt.float32
    bf16 = mybir.dt.bfloat16
    e3 = mybir.dt.float8e3
    Tanh = mybir.ActivationFunctionType.Tanh

    # ct columns: [w2c: NB*256 | wlc: NB*128 (rows 0:2) | bdt: TOT | bdl: TOT2]
    O_WLC = NB * 256
    O_BDT = O_WLC + NB * 128
    O_BDL = O_BDT + TOT
    CTW = O_BDL + TOT2

    nc = bacc.Bacc("TRN2", target_bir_lowering=False, debug=False,
                   num_devices=NCORES)

    xg_d = nc.dram_tensor("xg", [65, TOT], bf16, kind="ExternalInput")
    w1_d = nc.dram_tensor("w1", [65, SLOTS * 256], e3, kind="ExternalInput")
    w2_d = nc.dram_tensor("w2", [128, SLOTS * 512], e3, kind="ExternalInput")
    wl_d = nc.dram_tensor("wl", [128, SLOTS * 128], bf16, kind="ExternalInput")
    ct_d = nc.dram_tensor("ct", [8, CTW], bf16, kind="ExternalInput")
    out_d = nc.dram_tensor("out", [128, TOT2], bf16, kind="ExternalOutput")

    # block -> first/last slot columns for the per-2-block weight loads
    def slot_range(b0, b1):
        lo = blocks[b0][0]
        hi = blocks[b1][-1] + 1
        return lo, hi

    with tile.TileContext(nc) as tc:
        with (
            tc.tile_pool(name="stat", bufs=1) as spool,
            tc.tile_pool(name="acts", bufs=NB) as hpool,
            tc.tile_pool(name="ph", bufs=3, space=bass.MemorySpace.PSUM) as php,
            tc.tile_pool(name="po", bufs=2, space=bass.MemorySpace.PSUM) as pop,
        ):
            ct = spool.tile([8, CTW], bf16, tag="ct")
            xg = spool.tile([65, TOT], bf16, tag="xg")
            w1t = spool.tile([65, SLOTS * 256], e3, tag="w1")
            w2t = spool.tile([128, SLOTS * 512], e3, tag="w2")
            wlt = spool.tile([128, SLOTS * 128], bf16, tag="wl")
            ostage = spool.tile([128, TOT2], bf16, tag="out")

            # ---- load schedule -------------------------------------------
            # First the inputs block 0's L1 needs, then W2 per block (the L2
            # pacing tensor) interleaved with paired Wl loads.
            wlg = [(b0, min(b0 + 1, NB - 1)) for b0 in range(0, NB, 2)]

            def load(tile_ap, dram, lo, hi, mult):
                nc.sync.dma_start(tile_ap[:, lo * mult:hi * mult],
                                  dram.ap()[:, lo * mult:hi * mult])

            bhalf = min(2, NB - 1)
            xs = XO[bhalf] if bhalf < NB else TOT
            nc.sync.dma_start(xg[:, 0:xs], xg_d.ap()[:, 0:xs])
            w1half = (blocks[bhalf][0] if bhalf < NB else SLOTS) * 256
            nc.sync.dma_start(w1t[:, 0:w1half], w1_d.ap()[:, 0:w1half])
            nc.sync.dma_start(ct[:], ct_d.ap())
            load(w2t, w2_d, *slot_range(0, 0), 512)
            nc.sync.dma_start(xg[:, xs:], xg_d.ap()[:, xs:])
            nc.sync.dma_start(w1t[:, w1half:], w1_d.ap()[:, w1half:])
            rest, emitted = [], 0
            for b in range(1, NB):
                rest.append((w2t, w2_d, (b, b), 512))
                if b == 2 * emitted + 1 and emitted < len(wlg):
                    rest.append((wlt, wl_d, wlg[emitted], 128))
                    emitted += 1
            for gi in range(emitted, len(wlg)):
                rest.append((wlt, wl_d, wlg[gi], 128))
            for tile_ap, dram, grp, mult in rest:
                lo, hi = slot_range(*grp)
                load(tile_ap, dram, lo, hi, mult)

            # store group boundaries (blocks): roughly thirds
            sb1, sb2 = max(1, NB // 3), max(2, (2 * NB) // 3)
            store_after = {sb1 - 1: (0, sb1 - 1), sb2 - 1: (sb1, sb2 - 1),
                           NB - 1: (sb2, NB - 1)}

            # ---- software-pipelined compute ------------------------------
            # One PSUM tile per block serves both L1 and L2 (L2's start=True
            # rezeroes it after tanh1 has read it), so a 3-deep ring plus two
            # 1-bank L3 tiles fits the 8 PSUM banks without stalling.
            def emit_l1(b):
                bslots, Sb, xob = blocks[b], S[b], xoff[b]
                ph1 = php.tile([128, 1024], f32, tag="ph", name=f"ph_{b}")
                for t in range(2):
                    for i, s in enumerate(bslots):
                        c = caps[s]
                        nc.tensor.matmul(
                            ph1[:, t * 512 + xob[i]: t * 512 + xob[i] + c],
                            w1t[:, s * 256 + t * 128: s * 256 + (t + 1) * 128],
                            xg[:, XO[b] + xob[i]: XO[b] + xob[i] + c],
                        )
                h1 = hpool.tile([128, 2 * Sb], bf16, tag="h1",
                                padded_shape=[128, 2 * max(S)], name=f"h1_{b}")
                nc.scalar.activation(
                    h1[:].rearrange("p (t s) -> p t s", t=2),
                    ph1[:].rearrange("p (t s) -> p t s", t=2)[:, :, 0:Sb],
                    Tanh, scale=1.0 / WSCALE,
                )
                return h1, ph1

            def emit_l2(b, h1, ph2):
                bslots, Sb, xob = blocks[b], S[b], xoff[b]
                for t in range(2):
                    nc.tensor.matmul(
                        ph2[:, t * 512: t * 512 + Sb],
                        ct[:, (b * 2 + t) * 128: (b * 2 + t + 1) * 128],
                        ct[:, O_BDT + XO[b]: O_BDT + XO[b] + Sb],
                        start=True, stop=False, skip_group_check=True,
                    )
                    for i, s in enumerate(bslots):
                        c = caps[s]
                        last = i == len(bslots) - 1
                        nc.tensor.matmul(
                            ph2[:, t * 512 + xob[i]: t * 512 + xob[i] + c],
                            w2t[:, s * 512 + t * 128: s * 512 + (t + 1) * 128],
                            h1[:, xob[i]: xob[i] + c],
                            start=False, stop=False, skip_group_check=True,
                        )
                        nc.tensor.matmul(
                            ph2[:, t * 512 + xob[i]: t * 512 + xob[i] + c],
                            w2t[:, s * 512 + 256 + t * 128: s * 512 + 256 + (t + 1) * 128],
                            h1[:, Sb + xob[i]: Sb + xob[i] + c],
                            start=False, stop=last, skip_group_check=True,
                        )
                h2 = hpool.tile([128, 2 * Sb], bf16, tag="h2",
                                padded_shape=[128, 2 * max(S)], name=f"h2_{b}")
                nc.scalar.activation(
                    h2[:].rearrange("p (t s) -> p t s", t=2),
                    ph2[:].rearrange("p (t s) -> p t s", t=2)[:, :, 0:Sb],
                    Tanh, scale=1.0 / WSCALE,
                )
                return h2

            def emit_l3(b, h2):
                Sb, xob = S[b], xoff[b]
                po = pop.tile([128, PS[b]], f32, tag="po",
                              padded_shape=[128, max(PS)], name=f"po_{b}")
                for h in range(2):
                    nc.tensor.matmul(
                        po[h * 64:(h + 1) * 64, 0:PS[b]],
                        ct[0:4, O_WLC + b * 128 + h * 64: O_WLC + b * 128 + (h + 1) * 64],
                        ct[0:4, O_BDL + OO[b]: O_BDL + OO[b] + PS[b]],
                        start=True, stop=False, skip_group_check=True,
                    )
                nmm = sum(1 for pr in pairs[b] for s in pr if s is not None)
                k = 0
                for ci, (sa, sb_) in enumerate(pairs[b]):
                    for h, s in enumerate((sa, sb_)):
                        if s is None:
                            continue
                        c = caps[s]
                        xo = xob[2 * ci + h]
                        k += 1
                        nc.tensor.matmul(
                            po[h * 64:(h + 1) * 64,
                               poff[b][ci]: poff[b][ci] + c],
                            wlt[:, s * 128: s * 128 + 64],
                            h2[:, xo: xo + c],
                            start=False, stop=False, skip_group_check=True,
                        )
                        nc.tensor.matmul(
                            po[h * 64:(h + 1) * 64,
                               poff[b][ci]: poff[b][ci] + c],
                            wlt[:, s * 128 + 64: s * 128 + 128],
                            h2[:, Sb + xo: Sb + xo + c],
                            start=False, stop=k == nmm, skip_group_check=True,
                        )
                nc.vector.tensor_copy(ostage[:, OO[b]: OO[b] + PS[b]],
                                      po[:, 0:PS[b]])
                if b in store_after:
                    b0, b1 = store_after[b]
                    nc.gpsimd.dma_start(
                        out_d.ap()[:, OO[b0]: OO[b1] + PS[b1]],
                        ostage[:, OO[b0]: OO[b1] + PS[b1]],
                    )

            # software pipeline, PE order per iteration: L2(i), L3(i-1),
            # L1(i+2). The lookahead keeps every PSUM-ring WAR dependency one
            # full iteration old by the time it's needed, so the in-order PE
            # queue never stalls on ACT.
            h1s, h2s, phs = {}, {}, {}
            for i in range(min(2, NB)):
                h1s[i], phs[i] = emit_l1(i)
            for i in range(NB):
                h2s[i] = emit_l2(i, h1s.pop(i), phs.pop(i))
                if i >= 1:
                    emit_l3(i - 1, h2s.pop(i - 1))
                if i + 2 < NB:
                    h1s[i + 2], phs[i + 2] = emit_l1(i + 2)
            emit_l3(NB - 1, h2s.pop(NB - 1))

    nc.compile()
    return nc


def _plan(ind):
    counts = np.bincount(ind, minlength=NEXP)
    perm = np.argsort(-counts, kind="stable")
    caps = []
    for s in range(SLOTS):
        c = int(counts[perm[8 * s]])
        caps.append(0 if c == 0 else int(np.ceil(c / 2)) * 2)
    return counts, perm, caps


def _prep_inputs(x, ind, W1, b1, W2, b2, Wl, bl, perm, caps, g):
    """Build per-core arrays for the count-sorted round-robin layout."""
    blocks, S, XO, xoff = g["blocks"], g["S"], g["XO"], g["xoff"]
    pairs, poff, OO = g["pairs"], g["poff"], g["OO"]
    NB = len(blocks)
    TOT, TOT2 = g["TOT"], g["TOT2"]
    O_WLC = NB * 256
    O_BDT = O_WLC + NB * 128
    O_BDL = O_BDT + TOT
    CTW = O_BDL + TOT2

    order = np.argsort(ind, kind="stable")
    offs = np.zeros(NEXP + 1, np.int64)
    np.cumsum(np.bincount(ind, minlength=NEXP), out=offs[1:])
    rows = [order[offs[e]:offs[e + 1]] for e in range(NEXP)]

    # scaled transposed weights, shared across cores
    w1aug = np.concatenate([W1, b1[:, :, None]], axis=2)       # [E, 256, 65]
    w1q = (w1aug * WSCALE).astype(E3M4)                        # e3m4 x64
    w2q = (W2 * WSCALE).astype(E3M4)                           # [E, 256, 256]
    wlb = Wl.astype(BF16)                                      # [E, 64, 256]
    b2q = (b2 * WSCALE).astype(np.float32)
    xb = x.astype(BF16)

    in_maps = []
    for k in range(NCORES):
        xg = np.zeros((65, TOT), np.float32)
        w1 = np.zeros((65, SLOTS * 256), E3M4)
        w2 = np.zeros((128, SLOTS * 512), E3M4)
        wl = np.zeros((128, SLOTS * 128), np.float32)
        ct = np.zeros((8, CTW), np.float32)
        for b in range(NB):
            for i, s in enumerate(blocks[b]):
                if caps[s] == 0:
                    continue
                e = perm[8 * s + k]
                r = rows[e]
                col = XO[b] + xoff[b][i]
                xg[0:64, col: col + len(r)] = xb[r].astype(np.float32).T
                xg[64, col: col + caps[s]] = 1.0
                w1[:, s * 256:(s + 1) * 256] = w1q[e].T
                # W2 chunks A|B, each [128, 256]
                w2[:, s * 512: s * 512 + 256] = w2q[e, :, 0:128].T
                w2[:, s * 512 + 256: (s + 1) * 512] = w2q[e, :, 128:256].T
                wl[:, s * 128: s * 128 + 64] = wlb[e, :, 0:128].astype(np.float32).T
                wl[:, s * 128 + 64: (s + 1) * 128] = wlb[e, :, 128:256].astype(np.float32).T
                ct[i, b * 256:(b + 1) * 256] = b2q[e]
                ct[i, O_BDT + col: O_BDT + col + caps[s]] = 1.0
            for ci, (sa, sb_) in enumerate(pairs[b]):
                pc = OO[b] + poff[b][ci]
                w = max(caps[sa], caps[sb_] if sb_ is not None else 0)
                ct[ci, O_BDL + pc: O_BDL + pc + w] = 1.0
                for h, s in enumerate((sa, sb_)):
                    if s is None or caps[s] == 0:
                        continue
                    e = perm[8 * s + k]
                    ct[ci, O_WLC + b * 128 + h * 64: O_WLC + b * 128 + (h + 1) * 64] = bl[e]
        in_maps.append({
            "xg": xg.astype(BF16),
            "w1": w1,
            "w2": w2,
            "wl": wl.astype(BF16),
            "ct": ct.astype(BF16),
        })
    return in_maps, rows


def _unscatter(results, rows, perm, caps, g):
    blocks, xoff, poff, OO, pairs = g["blocks"], g["xoff"], g["poff"], g["OO"], g["pairs"]
    out = np.empty((N, LIN), np.float32)
    for k in range(NCORES):
        arr = np.asarray(results[k]["out"], np.float32)
        for b in range(len(blocks)):
            for ci, (sa, sb_) in enumerate(pairs[b]):
                for h, s in enumerate((sa, sb_)):
                    if s is None or caps[s] == 0:
                        continue
                    e = perm[8 * s + k]
                    r = rows[e]
                    col = OO[b] + poff[b][ci]
                    out[r, :] = arr[h * 64:(h + 1) * 64, col: col + len(r)].T
    return out


def kernel(x, ind, W1, b1, W2, b2, Wl, bl):
    from concourse.bass_utils import run_bass_kernel_spmd

    x = np.asarray(x, np.float32)
    ind = np.asarray(ind).astype(np.int64)
    W1 = np.asarray(W1, np.float32); b1 = np.asarray(b1, np.float32)
    W2 = np.asarray(W2, np.float32); b2 = np.asarray(b2, np.float32)
    Wl = np.asarray(Wl, np.float32); bl = np.asarray(bl, np.float32)

    counts, perm, caps = _plan(ind)
    g = _geometry(caps)

    key = tuple(caps)
    if key not in _cache:
        _cache[key] = _build_program(caps)
    nc = _cache[key]

    in_maps, rows = _prep_inputs(x, ind, W1, b1, W2, b2, Wl, bl, perm, caps, g)
    res = run_bass_kernel_spmd(nc, in_maps, core_ids=list(range(NCORES)))
    return _unscatter(res.results, rows, perm, caps, g)


# revision 21
# speedup vs baseline: 1.0544x; 1.0544x over previous
"""MoE routing kernel (nn_JSMLP): per-row expert-indexed 3-layer MLP.

  out[n] = Wl[i] @ tanh(W2[i] @ tanh(W1[i] @ x[n] + b1[i]) + b2[i]) + bl[i],  i = ind[n]

Sharding (hardcoded): expert-parallel across 8 cores, load-balanced by
count-sorted round-robin so a single SPMD program fits all cores:
experts are sorted by row count (desc); rank r goes to core r%8, slot r//8.
Slot s then has the same capacity cap[s] = roundup(count of rank 8s, 4) on
every core, so per-slot column spans are compile-time constants while padding
stays ~3% (vs ~50% for a uniform max-count capacity).

Numerics: W1 (with b1 row) and W2 are stored as float8_e3m4 scaled by 64
(values land in e3m4's normal range; 4 mantissa bits ~ 1.2% rms/elem); the
1/64 descale rides the tanh activations for free (out = tanh(scale*in)).
Wl, x, h, biases stay bf16; PSUM accumulates fp32. Measured end-to-end rel
err ~1.7e-2 vs the fp32 reference (gate: 2e-2).

Per core, per block b (4 slots, S_b = sum of caps <= 512 = one PSUM bank):
  L1: H1T[256, S_b] = W1augT.T @ [x;1]T   (bias via ones-row, K=65, e3m4 x64)
  L2: H2T[256, S_b] = W2T.T @ tanh(H1T/64)  (K=256 in 2 chunks; b2 seeded by a
      tiny block-diag ones matmul in bf16, also x64)
  L3: outT[2x64, PS_b] = WlT.T @ tanh(H2T/64)  (2 experts per 128-partition
      tile; bl seeded via pair-diag ones; plain bf16)

DMA plan (the shared DMA engine pool is the roofline at ~360 B/ns): one load
each for consts/x/W1 (W1 split in 2), per-2-block loads for W2/Wl, per-2-block
stores on the DVE queue; 15 DMAs total keeps the serial HWDGE (~630ns each)
off the critical path. All loads are issued into resident SBUF tiles (no ring
reuse) so the load queue never blocks on consumers.
"""

import numpy as np
import ml_dtypes

N, IN_DIM, H1, H2, LIN, NEXP = 16384, 64, 256, 256, 64, 256
NCORES = 8
SLOTS = NEXP // NCORES  # 32 experts per core

BF16 = ml_dtypes.bfloat16
E3M4 = ml_dtypes.float8_e3m4
WSCALE = 64.0

_cache = {}


def _geometry(caps):
    """Block/pair geometry shared by program builder and host prep.

    caps: per-slot capacities (len 32, multiples of 4, may be 0).
    Returns dict with blocks (list of slot-index lists), per-block slot
    offsets, block x-offsets, pair layout and output offsets.
    """
    blocks = []
    cur, cur_sum = [], 0
    for s in range(SLOTS):
        c = caps[s]
        if c == 0:
            continue
        if len(cur) == 8 or (cur_sum + c > 512 and cur):
            blocks.append(cur)
            cur, cur_sum = [], 0
        cur.append(s)
        cur_sum += c
    if cur:
        blocks.append(cur)
    # tiny trailing blocks shorten the drain after the last weight DMA
    if len(blocks[-1]) > 1:
        blocks.append([blocks[-1].pop()])
    if len(blocks) > 1 and len(blocks[-2]) > 1:
        blocks.insert(len(blocks) - 1, [blocks[-2].pop()])
    # pyramid execution order: a small block first (quick pipeline fill and
    # PE p-state warm-up), big blocks mid-stream, tiny blocks last (short
    # drain after the final weight DMA)
    if len(blocks) > 3:
        tail, normals = blocks[-2:], blocks[:-2]
        ns = sorted(normals, key=lambda bl: sum(caps[s] for s in bl))
        blocks = ns[0::2] + ns[1::2][::-1] + tail

    g = {"blocks": blocks, "xoff": [], "S": [], "XO": [], "pairs": [],
         "poff": [], "PS": [], "OO": []}
    xo_total, oo_total = 0, 0
    for bl in blocks:
        offs, acc = [], 0
        for s in bl:
            offs.append(acc)
            acc += caps[s]
        assert acc <= 512, f"block span {acc} exceeds a PSUM bank"
        g["xoff"].append(offs)
        g["S"].append(acc)
        g["XO"].append(xo_total)
        xo_total += acc
        prs = [(bl[i], bl[i + 1] if i + 1 < len(bl) else None)
               for i in range(0, len(bl), 2)]
        poffs, pacc = [], 0
        for a, b in prs:
            poffs.append(pacc)
            pacc += max(caps[a], caps[b] if b is not None else 0)
        g["pairs"].append(prs)
        g["poff"].append(poffs)
        g["PS"].append(pacc)
        g["OO"].append(oo_total)
        oo_total += pacc
    g["TOT"] = xo_total
    g["TOT2"] = oo_total
    # weight tensors are laid out in execution order: slot s lives at
    # column-group scol[s], so per-block loads are contiguous slices
    scol, idx = {}, 0
    for bl in blocks:
        for s in bl:
            scol[s] = idx
            idx += 1
    g["scol"] = scol
    return g


def _build_program(caps):
    import concourse.bass as bass
    import concourse.tile as tile
    from concourse import bacc, mybir

    caps = list(caps)
    g = _geometry(caps)
    blocks, S, XO, xoff = g["blocks"], g["S"], g["XO"], g["xoff"]
    pairs, poff, PS, OO = g["pairs"], g["poff"], g["PS"], g["OO"]
    NB = len(blocks)
    TOT, TOT2 = g["TOT"], g["TOT2"]
    scol = g["scol"]

    f32 = mybir.dt.float32
    bf16 = mybir.dt.bfloat16
    e3 = mybir.dt.float8e3
    Tanh = mybir.ActivationFunctionType.Tanh

    # ct columns: [w2c: NB*256 | wlc: NB*128 (rows 0:2) | bdt: TOT | bdl: TOT2]
    O_WLC = NB * 256
    O_BDT = O_WLC + NB * 128
    O_BDL = O_BDT + TOT
    CTW = O_BDL + TOT2

    nc = bacc.Bacc("TRN2", target_bir_lowering=False, debug=False,
                   num_devices=NCORES)

    xg_d = nc.dram_tensor("xg", [65, TOT], bf16, kind="ExternalInput")
    w1_d = nc.dram_tensor("w1", [65, SLOTS * 256], e3, kind="ExternalInput")
    w2_d = nc.dram_tensor("w2", [128, SLOTS * 512], e3, kind="ExternalInput")
    wl_d = nc.dram_tensor("wl", [128, SLOTS * 128], bf16, kind="ExternalInput")
    ct_d = nc.dram_tensor("ct", [8, CTW], bf16, kind="ExternalInput")
    out_d = nc.dram_tensor("out", [128, TOT2], bf16, kind="ExternalOutput")

    # block -> first/last weight-column groups (exec-order layout)
    def slot_range(b0, b1):
        lo = scol[blocks[b0][0]]
        hi = scol[blocks[b1][-1]] + 1
        return lo, hi

    with tile.TileContext(nc) as tc:
        with (
            tc.tile_pool(name="stat", bufs=1) as spool,
            tc.tile_pool(name="acts", bufs=NB) as hpool,
            tc.tile_pool(name="ph", bufs=3, space=bass.MemorySpace.PSUM) as php,
            tc.tile_pool(name="po", bufs=2, space=bass.MemorySpace.PSUM) as pop,
        ):
            ct = spool.tile([8, CTW], bf16, tag="ct")
            xg = spool.tile([65, TOT], bf16, tag="xg")
            w1t = spool.tile([65, SLOTS * 256], e3, tag="w1")
            w2t = spool.tile([128, SLOTS * 512], e3, tag="w2")
            wlt = spool.tile([128, SLOTS * 128], bf16, tag="wl")
            ostage = spool.tile([128, TOT2], bf16, tag="out")

            # ---- load schedule -------------------------------------------
            # First the inputs block 0's L1 needs, then W2 per block (the L2
            # pacing tensor) interleaved with paired Wl loads.
            wlg = [(b0, min(b0 + 1, NB - 1)) for b0 in range(0, NB, 2)]

            def load(tile_ap, dram, lo, hi, mult):
                nc.sync.dma_start(tile_ap[:, lo * mult:hi * mult],
                                  dram.ap()[:, lo * mult:hi * mult])

            bhalf = min(2, NB - 1)
            xs = XO[bhalf] if bhalf < NB else TOT
            nc.sync.dma_start(xg[:, 0:xs], xg_d.ap()[:, 0:xs])
            w1half = (scol[blocks[bhalf][0]] if bhalf < NB else SLOTS) * 256
            nc.sync.dma_start(w1t[:, 0:w1half], w1_d.ap()[:, 0:w1half])
            nc.sync.dma_start(ct[:], ct_d.ap())
            load(w2t, w2_d, *slot_range(0, 0), 512)
            nc.sync.dma_start(xg[:, xs:], xg_d.ap()[:, xs:])
            nc.sync.dma_start(w1t[:, w1half:], w1_d.ap()[:, w1half:])
            rest, emitted = [], 0
            for b in range(1, NB):
                rest.append((w2t, w2_d, (b, b), 512))
                if b == 2 * emitted + 1 and emitted < len(wlg):
                    rest.append((wlt, wl_d, wlg[emitted], 128))
                    emitted += 1
            for gi in range(emitted, len(wlg)):
                rest.append((wlt, wl_d, wlg[gi], 128))
            for tile_ap, dram, grp, mult in rest:
                lo, hi = slot_range(*grp)
                load(tile_ap, dram, lo, hi, mult)

            # store group boundaries (blocks): roughly thirds
            sb1, sb2 = max(1, NB // 3), max(2, (2 * NB) // 3)
            store_after = {sb1 - 1: (0, sb1 - 1), sb2 - 1: (sb1, sb2 - 1),
                           NB - 1: (sb2, NB - 1)}

            # ---- software-pipelined compute ------------------------------
            # One PSUM tile per block serves both L1 and L2 (L2's start=True
            # rezeroes it after tanh1 has read it), so a 3-deep ring plus two
            # 1-bank L3 tiles fits the 8 PSUM banks without stalling.
            def emit_l1(b):
                bslots, Sb, xob = blocks[b], S[b], xoff[b]
                ph1 = php.tile([128, 1024], f32, tag="ph", name=f"ph_{b}")
                for t in range(2):
                    for i, s in enumerate(bslots):
                        c = caps[s]
                        nc.tensor.matmul(
                            ph1[:, t * 512 + xob[i]: t * 512 + xob[i] + c],
                            w1t[:, scol[s] * 256 + t * 128: scol[s] * 256 + (t + 1) * 128],
                            xg[:, XO[b] + xob[i]: XO[b] + xob[i] + c],
                        )
                h1 = hpool.tile([128, 2 * Sb], bf16, tag="h1",
                                padded_shape=[128, 2 * max(S)], name=f"h1_{b}")
                nc.scalar.activation(
                    h1[:].rearrange("p (t s) -> p t s", t=2),
                    ph1[:].rearrange("p (t s) -> p t s", t=2)[:, :, 0:Sb],
                    Tanh, scale=1.0 / WSCALE,
                )
                return h1, ph1

            def emit_l2(b, h1, ph2):
                bslots, Sb, xob = blocks[b], S[b], xoff[b]
                for t in range(2):
                    nc.tensor.matmul(
                        ph2[:, t * 512: t * 512 + Sb],
                        ct[:, (b * 2 + t) * 128: (b * 2 + t + 1) * 128],
                        ct[:, O_BDT + XO[b]: O_BDT + XO[b] + Sb],
                        start=True, stop=False, skip_group_check=True,
                    )
                    for i, s in enumerate(bslots):
                        c = caps[s]
                        last = i == len(bslots) - 1
                        nc.tensor.matmul(
                            ph2[:, t * 512 + xob[i]: t * 512 + xob[i] + c],
                            w2t[:, scol[s] * 512 + t * 128: scol[s] * 512 + (t + 1) * 128],
                            h1[:, xob[i]: xob[i] + c],
                            start=False, stop=False, skip_group_check=True,
                        )
                        nc.tensor.matmul(
                            ph2[:, t * 512 + xob[i]: t * 512 + xob[i] + c],
                            w2t[:, scol[s] * 512 + 256 + t * 128: scol[s] * 512 + 256 + (t + 1) * 128],
                            h1[:, Sb + xob[i]: Sb + xob[i] + c],
                            start=False, stop=last, skip_group_check=True,
                        )
                h2 = hpool.tile([128, 2 * Sb], bf16, tag="h2",
                                padded_shape=[128, 2 * max(S)], name=f"h2_{b}")
                nc.scalar.activation(
                    h2[:].rearrange("p (t s) -> p t s", t=2),
                    ph2[:].rearrange("p (t s) -> p t s", t=2)[:, :, 0:Sb],
                    Tanh, scale=1.0 / WSCALE,
                )
                return h2

            def emit_l3(b, h2):
                Sb, xob = S[b], xoff[b]
                po = pop.tile([128, PS[b]], f32, tag="po",
                              padded_shape=[128, max(PS)], name=f"po_{b}")
                for h in range(2):
                    nc.tensor.matmul(
                        po[h * 64:(h + 1) * 64, 0:PS[b]],
                        ct[0:4, O_WLC + b * 128 + h * 64: O_WLC + b * 128 + (h + 1) * 64],
                        ct[0:4, O_BDL + OO[b]: O_BDL + OO[b] + PS[b]],
                        start=True, stop=False, skip_group_check=True,
                    )
                nmm = sum(1 for pr in pairs[b] for s in pr if s is not None)
                k = 0
                for ci, (sa, sb_) in enumerate(pairs[b]):
                    for h, s in enumerate((sa, sb_)):
                        if s is None:
                            continue
                        c = caps[s]
                        xo = xob[2 * ci + h]
                        k += 1
                        nc.tensor.matmul(
                            po[h * 64:(h + 1) * 64,
                               poff[b][ci]: poff[b][ci] + c],
                            wlt[:, scol[s] * 128: scol[s] * 128 + 64],
                            h2[:, xo: xo + c],
                            start=False, stop=False, skip_group_check=True,
                        )
                        nc.tensor.matmul(
                            po[h * 64:(h + 1) * 64,
                               poff[b][ci]: poff[b][ci] + c],
                            wlt[:, scol[s] * 128 + 64: scol[s] * 128 + 128],
                            h2[:, Sb + xo: Sb + xo + c],
                            start=False, stop=k == nmm, skip_group_check=True,
                        )
                nc.vector.tensor_copy(ostage[:, OO[b]: OO[b] + PS[b]],
                                      po[:, 0:PS[b]])
                if b in store_after:
                    b0, b1 = store_after[b]
                    nc.gpsimd.dma_start(
                        out_d.ap()[:, OO[b0]: OO[b1] + PS[b1]],
                        ostage[:, OO[b0]: OO[b1] + PS[b1]],
                    )

            # software pipeline, PE order per iteration: L2(i), L3(i-1),
            # L1(i+2). The lookahead keeps every PSUM-ring WAR dependency one
            # full iteration old by the time it's needed, so the in-order PE
            # queue never stalls on ACT.
            h1s, h2s, phs = {}, {}, {}
            for i in range(min(2, NB)):
                h1s[i], phs[i] = emit_l1(i)
            for i in range(NB):
                h2s[i] = emit_l2(i, h1s.pop(i), phs.pop(i))
                if i >= 1:
                    emit_l3(i - 1, h2s.pop(i - 1))
                if i + 2 < NB:
                    h1s[i + 2], phs[i + 2] = emit_l1(i + 2)
            emit_l3(NB - 1, h2s.pop(NB - 1))

    nc.compile()
    return nc


def _plan(ind):
    counts = np.bincount(ind, minlength=NEXP)
    perm = np.argsort(-counts, kind="stable")
    caps = []
    for s in range(SLOTS):
        c = int(counts[perm[8 * s]])
        caps.append(0 if c == 0 else int(np.ceil(c / 2)) * 2)
    return counts, perm, caps


def _prep_inputs(x, ind, W1, b1, W2, b2, Wl, bl, perm, caps, g):
    """Build per-core arrays for the count-sorted round-robin layout."""
    blocks, S, XO, xoff = g["blocks"], g["S"], g["XO"], g["xoff"]
    pairs, poff, OO = g["pairs"], g["poff"], g["OO"]
    scol = g["scol"]
    NB = len(blocks)
    TOT, TOT2 = g["TOT"], g["TOT2"]
    O_WLC = NB * 256
    O_BDT = O_WLC + NB * 128
    O_BDL = O_BDT + TOT
    CTW = O_BDL + TOT2

    order = np.argsort(ind, kind="stable")
    offs = np.zeros(NEXP + 1, np.int64)
    np.cumsum(np.bincount(ind, minlength=NEXP), out=offs[1:])
    rows = [order[offs[e]:offs[e + 1]] for e in range(NEXP)]

    # scaled transposed weights, shared across cores
    w1aug = np.concatenate([W1, b1[:, :, None]], axis=2)       # [E, 256, 65]
    w1q = (w1aug * WSCALE).astype(E3M4)                        # e3m4 x64
    w2q = (W2 * WSCALE).astype(E3M4)                           # [E, 256, 256]
    wlb = Wl.astype(BF16)                                      # [E, 64, 256]
    b2q = (b2 * WSCALE).astype(np.float32)
    xb = x.astype(BF16)

    in_maps = []
    for k in range(NCORES):
        xg = np.zeros((65, TOT), np.float32)
        w1 = np.zeros((65, SLOTS * 256), E3M4)
        w2 = np.zeros((128, SLOTS * 512), E3M4)
        wl = np.zeros((128, SLOTS * 128), np.float32)
        ct = np.zeros((8, CTW), np.float32)
        for b in range(NB):
            for i, s in enumerate(blocks[b]):
                if caps[s] == 0:
                    continue
                e = perm[8 * s + k]
                r = rows[e]
                col = XO[b] + xoff[b][i]
                xg[0:64, col: col + len(r)] = xb[r].astype(np.float32).T
                xg[64, col: col + caps[s]] = 1.0
                cs = scol[s]
                w1[:, cs * 256:(cs + 1) * 256] = w1q[e].T
                # W2 chunks A|B, each [128, 256]
                w2[:, cs * 512: cs * 512 + 256] = w2q[e, :, 0:128].T
                w2[:, cs * 512 + 256: (cs + 1) * 512] = w2q[e, :, 128:256].T
                wl[:, cs * 128: cs * 128 + 64] = wlb[e, :, 0:128].astype(np.float32).T
                wl[:, cs * 128 + 64: (cs + 1) * 128] = wlb[e, :, 128:256].astype(np.float32).T
                ct[i, b * 256:(b + 1) * 256] = b2q[e]
                ct[i, O_BDT + col: O_BDT + col + caps[s]] = 1.0
            for ci, (sa, sb_) in enumerate(pairs[b]):
                pc = OO[b] + poff[b][ci]
                w = max(caps[sa], caps[sb_] if sb_ is not None else 0)
                ct[ci, O_BDL + pc: O_BDL + pc + w] = 1.0
                for h, s in enumerate((sa, sb_)):
                    if s is None or caps[s] == 0:
                        continue
                    e = perm[8 * s + k]
                    ct[ci, O_WLC + b * 128 + h * 64: O_WLC + b * 128 + (h + 1) * 64] = bl[e]
        in_maps.append({
            "xg": xg.astype(BF16),
            "w1": w1,
            "w2": w2,
            "wl": wl.astype(BF16),
            "ct": ct.astype(BF16),
        })
    return in_maps, rows


def _unscatter(results, rows, perm, caps, g):
    blocks, xoff, poff, OO, pairs = g["blocks"], g["xoff"], g["poff"], g["OO"], g["pairs"]
    out = np.empty((N, LIN), np.float32)
    for k in range(NCORES):
        arr = np.asarray(results[k]["out"], np.float32)
        for b in range(len(blocks)):
            for ci, (sa, sb_) in enumerate(pairs[b]):
                for h, s in enumerate((sa, sb_)):
                    if s is None or caps[s] == 0:
                        continue
                    e = perm[8 * s + k]
                    r = rows[e]
                    col = OO[b] + poff[b][ci]
                    out[r, :] = arr[h * 64:(h + 1) * 64, col: col + len(r)].T
    return out


def kernel(x, ind, W1, b1, W2, b2, Wl, bl):
    from concourse.bass_utils import run_bass_kernel_spmd

    x = np.asarray(x, np.float32)
    ind = np.asarray(ind).astype(np.int64)
    W1 = np.asarray(W1, np.float32); b1 = np.asarray(b1, np.float32)
    W2 = np.asarray(W2, np.float32); b2 = np.asarray(b2, np.float32)
    Wl = np.asarray(Wl, np.float32); bl = np.asarray(bl, np.float32)

    counts, perm, caps = _plan(ind)
    g = _geometry(caps)

    key = tuple(caps)
    if key not in _cache:
        _cache[key] = _build_program(caps)
    nc = _cache[key]

    in_maps, rows = _prep_inputs(x, ind, W1, b1, W2, b2, Wl, bl, perm, caps, g)
    res = run_bass_kernel_spmd(nc, in_maps, core_ids=list(range(NCORES)))
    return _unscatter(res.results, rows, perm, caps, g)
